# revision 1
# baseline (speedup 1.0000x reference)
"""Multi-head attention (b=8, n=1024, dim=1024, 16 heads) on 8 TRN2 NeuronCores.

Data-parallel: one batch element per core. Each core runs an identical
Bass/Tile program computing qkv projection, softmax attention, and the
output projection for its [1024, 1024] slice, in bf16 with fp32 PSUM
accumulation.

Layout choices (host pre-transposes so the device never transposes):
  - xt   [c, n]   = x[i].T                       (bf16)
  - wqkt [c, 2h*d] = permuted q/k weights^T: head-pair p occupies
        f-tiles 2p (q rows of heads 2p,2p+1) and 2p+1 (k rows).
        A 128-row f-tile = [head 2p (64 rows); head 2p+1 (64 rows)], so
        the qkv matmul directly yields q^T/k^T pair tiles where the even
        head lives on partitions 0-63 and the odd head on 64-127.
  - wvt  [c, h*d] = wv.T, wpt [c, o] = w_proj.T  (bf16)

Per core:
  V    = x @ wv^T          -> SBUF [n, h*65] with a ones column per head
  qk^T = wqk_perm @ x^T    -> SBUF pair tiles [128, n]
  S^T  = k_h @ q_h^T       -> PSUM [nk_tile, nq]   (K=64 row-tiled pairs)
  attn^T = exp(0.125*S^T)  -> SBUF bf16 (ScalarE; no max subtraction --
           scores ~ N(0,1), exp stays well inside fp32/bf16 range, and
           softmax is shift-invariant so the result matches jax.nn.softmax)
  out^T_aug = V_aug^T @ attn^T -> PSUM [65, nq]; row 64 = softmax denom
  out^T = out^T_aug[0:64] * (1/denom)  (DVE mul; reciprocal row replicated
         across partitions with a GPSIMD partition_broadcast)
  y    = out_heads @ w_proj^T + b      (bias added on DVE during PSUM
         evacuation, from a DMA partition-broadcast bias tile)

S^T matmuls for the even/odd heads of a pair run as K=64 row-tiles at PE
row groups (0,0)/(64,0) — adjacent instructions with disjoint row groups
execute concurrently on the 128x128 array.

All inputs are packed into one [dim, 5120] bf16 DRAM tensor so the whole
input loads with 8 large DMAs (per-DMA issue overhead on this stack is
~0.6-1 us plus ~0.9 us semaphore propagation, so DMA count matters).
"""

import numpy as np
import ml_dtypes

B, N, DIM = 8, 1024, 1024
H, D = 16, 64
NP = 128  # partitions
NCHUNK = 512  # matmul free-dim chunk (one PSUM bank of fp32)
CT = DIM // NP  # 8 contraction chunks
NT = N // NP  # 8 n-tiles
NQC = N // NCHUNK  # 2 nq chunks
PAIRS = H // 2  # 8 head pairs

BF16 = ml_dtypes.bfloat16

_CACHE = {}


def build(loop_iters=1):
    """Build and compile the per-core Bacc graph. Cached per loop_iters."""
    if loop_iters in _CACHE:
        return _CACHE[loop_iters]

    import concourse.mybir as mybir
    import concourse.tile as tile
    from concourse import bacc

    fp32 = mybir.dt.float32
    bf16 = mybir.dt.bfloat16
    Exp = mybir.ActivationFunctionType.Exp

    nc = bacc.Bacc("TRN2", target_bir_lowering=False, debug=False, num_devices=8)

    PACK = N + 2 * DIM + DIM + DIM  # xt | wqkt | wvt | wpt along free dim
    inp = nc.declare_dram_parameter("inp", [DIM, PACK], bf16, isOutput=False)
    bias = nc.declare_dram_parameter("bias", [1, DIM], bf16, isOutput=False)
    out = nc.declare_dram_parameter("out", [N, DIM], fp32, isOutput=True)

    with tile.TileContext(nc) as tc:
        with (
            tc.tile_pool(name="weights", bufs=1) as wpool,
            tc.tile_pool(name="acts", bufs=1) as apool,
            tc.tile_pool(name="attn", bufs=18) as attnpool,
            tc.tile_pool(name="small", bufs=2) as spool,
            tc.tile_pool(name="big_ps", bufs=2, space="PSUM") as big_ps,
            tc.tile_pool(name="half_ps", bufs=4, space="PSUM") as half_ps,
        ):
            # ---- persistent SBUF tensors (loaded once, one DMA per c-chunk) ----
            packed_sb = wpool.tile([NP, CT, PACK], bf16, tag="packed")

            HOT = N + 4 * NP  # xt + wqkt f-tiles of pairs 0 and 1
            dma_engines = [nc.sync, nc.scalar, nc.gpsimd]
            for ct in range(CT):
                eng = dma_engines[ct % 3]
                eng.dma_start(packed_sb[:, ct, 0:HOT],
                              inp[ct * NP:(ct + 1) * NP, 0:HOT])
            for ct in range(CT):
                eng = dma_engines[ct % 3]
                eng.dma_start(packed_sb[:, ct, HOT:],
                              inp[ct * NP:(ct + 1) * NP, HOT:])
            bias_bc = wpool.tile([NP, DIM], bf16, tag="biasbc")
            nc.sync.dma_start(bias_bc[:], bias[0:1, :].to_broadcast((NP, DIM)))
            xt_sb = packed_sb[:, :, 0:N]
            wqkt_sb = packed_sb[:, :, N:N + 2 * DIM]
            wvt_sb = packed_sb[:, :, N + 2 * DIM:N + 3 * DIM]
            wpt_sb = packed_sb[:, :, N + 3 * DIM:N + 4 * DIM]

            def body(_it=None):
                # ---- per-iteration SBUF ----
                q_sb = apool.tile([NP, PAIRS, N], bf16, tag="q")
                k_sb = apool.tile([NP, PAIRS, N], bf16, tag="k")
                vaug_sb = apool.tile([NP, NT, H * (D + 1)], bf16, tag="vaug")
                outT_sb = apool.tile([NP, CT, N], bf16, tag="outT")

                def emit_qkv_ft(p, which, nqc=None):
                    # one f-tile of pair p's qk^T: which=0 -> q, 1 -> k.
                    # each nq half uses its own 1-bank psum tile so the hot
                    # S-tile rotation is starved for as little as possible.
                    ft = 2 * p + which
                    dst = q_sb if which == 0 else k_sb
                    nqcs = range(NQC) if nqc is None else (nqc,)
                    for nqc_ in nqcs:
                        qk_ps = big_ps.tile([NP, NCHUNK], fp32, tag="big",
                                            name="qk_ps")
                        for ct in range(CT):
                            nc.tensor.matmul(
                                qk_ps[:],
                                lhsT=wqkt_sb[:, ct, ft * NP:(ft + 1) * NP],
                                rhs=xt_sb[:, ct, nqc_ * NCHUNK:(nqc_ + 1) * NCHUNK],
                                start=(ct == 0),
                                stop=(ct == CT - 1),
                            )
                        nc.vector.tensor_copy(
                            dst[:, p, nqc_ * NCHUNK:(nqc_ + 1) * NCHUNK], qk_ps[:])

                def emit_qkv(p):
                    emit_qkv_ft(p, 0)
                    emit_qkv_ft(p, 1)

                def emit_v_tile(nt):
                    v_ps = big_ps.tile([NP, 2 * NCHUNK], fp32, tag="big",
                                       name="v_ps")
                    for ct in range(CT):
                        for fc in range(2):
                            nc.tensor.matmul(
                                v_ps[:, fc * NCHUNK:(fc + 1) * NCHUNK],
                                lhsT=xt_sb[:, ct, nt * NP:(nt + 1) * NP],
                                rhs=wvt_sb[:, ct, fc * NCHUNK:(fc + 1) * NCHUNK],
                                start=(ct == 0),
                                stop=(ct == CT - 1),
                            )
                    vrow = vaug_sb[:, nt, :].rearrange("p (h e) -> p h e", e=D + 1)
                    nc.vector.memset(vrow[:, :, D:D + 1], 1.0)
                    nc.vector.tensor_copy(
                        vrow[:, :, 0:D],
                        v_ps[:].rearrange("p (h e) -> p h e", e=D),
                    )

                def emit_s_exp(p, nkt, atn):
                    # S^T for both heads of pair p at nk-tile nkt; even head
                    # on PE rows 0-63, odd on 64-127 (row-tiled, concurrent)
                    sps = {}
                    for hh in range(2):
                        sps[hh] = big_ps.tile([NP, 2 * NCHUNK], fp32,
                                              tag="big", name="s_ps")
                    for nqc in range(NQC):
                        for hh in range(2):
                            lo, hi = hh * D, (hh + 1) * D
                            nc.tensor.matmul(
                                sps[hh][:, nqc * NCHUNK:(nqc + 1) * NCHUNK],
                                lhsT=k_sb[lo:hi, p, nkt * NP:(nkt + 1) * NP],
                                rhs=q_sb[lo:hi, p, nqc * NCHUNK:(nqc + 1) * NCHUNK],
                                start=True,
                                stop=True,
                                tile_position=(hh * D, 0),
                            )
                    for hh in range(2):
                        a = attnpool.tile([NP, N], bf16, tag="attn")
                        nc.scalar.activation(a[:], sps[hh][:], Exp,
                                             scale=float(D) ** -0.5)
                        atn[hh, nkt] = a

                # ---- software pipeline over head pairs ----
                # pair 0: S/exp interleaved with V tiles; then for each pair p:
                # S(p+1)/exp(p+1) interleaved per-nkt with AV(p), so ACT (the
                # attention-phase bottleneck) never starves.
                emit_qkv(0)
                atn_cur = {}
                for nkt in range(NT):
                    emit_s_exp(0, nkt, atn_cur)
                emit_qkv(1)

                for p in range(PAIRS):
                    av = {}
                    for hh in range(2):
                        for nqc in range(NQC):
                            av[hh, nqc] = half_ps.tile([D + 1, NCHUNK], fp32,
                                                       tag="half", name="av")
                    atn_nxt = {}
                    for nkt in range(NT):
                        if p + 1 < PAIRS:
                            emit_s_exp(p + 1, nkt, atn_nxt)
                        if p == 0 and nkt < 5:
                            emit_v_tile(nkt)
                        if p == 0 and nkt < 3:
                            emit_v_tile(5 + nkt)
                        if p + 2 < PAIRS and nkt in (1, 3, 5, 7):
                            emit_qkv_ft(p + 2, nkt // 4, (nkt % 4) // 2)
                        for hh in range(2):
                            h = 2 * p + hh
                            for nqc in range(NQC):
                                nc.tensor.matmul(
                                    av[hh, nqc][:],
                                    lhsT=vaug_sb[:, nkt,
                                                 h * (D + 1):(h + 1) * (D + 1)],
                                    rhs=atn_cur[hh, nkt][
                                        :, nqc * NCHUNK:(nqc + 1) * NCHUNK],
                                    start=(nkt == 0),
                                    stop=(nkt == NT - 1),
                                )
                    atn_cur = atn_nxt

                    for hh in range(2):
                        for nqc in range(NQC):
                            t = av[hh, nqc]
                            recip = spool.tile([1, NCHUNK], fp32, tag="recip")
                            nc.vector.reciprocal(recip[:], t[D:D + 1, :])
                            recip_b = spool.tile([D, NCHUNK], fp32, tag="recipb")
                            nc.gpsimd.partition_broadcast(recip_b[:], recip[:],
                                                          channels=D)
                            dst = outT_sb[hh * D:(hh + 1) * D, p,
                                          nqc * NCHUNK:(nqc + 1) * NCHUNK]
                            if hh == 0:
                                nc.vector.tensor_mul(dst, t[0:D, :], recip_b[:])
                            else:
                                tmp = spool.tile([D, NCHUNK], bf16, tag="tmpodd")
                                nc.vector.tensor_mul(tmp[:], t[0:D, :], recip_b[:])
                                # partition shift 0:64 -> 64:128 via DMA
                                nc.sync.dma_start(dst, tmp[:])

                # ---- y = out_heads @ w_proj^T + bias ----
                for nt in range(NT):
                    y_ps = {}
                    for oc in range(NQC):
                        y_ps[oc] = half_ps.tile([NP, NCHUNK], fp32, tag="half",
                                                name="y_ps")
                    for ct in range(CT):
                        for oc in range(NQC):
                            nc.tensor.matmul(
                                y_ps[oc][:],
                                lhsT=outT_sb[:, ct, nt * NP:(nt + 1) * NP],
                                rhs=wpt_sb[:, ct, oc * NCHUNK:(oc + 1) * NCHUNK],
                                start=(ct == 0),
                                stop=(ct == CT - 1),
                            )
                    y_sb = spool.tile([NP, N], fp32, tag="ysb", name="y_sb")
                    for oc in range(NQC):
                        nc.vector.tensor_add(
                            y_sb[:, oc * NCHUNK:(oc + 1) * NCHUNK], y_ps[oc][:],
                            bias_bc[:, oc * NCHUNK:(oc + 1) * NCHUNK],
                        )
                    nc.sync.dma_start(out[nt * NP:(nt + 1) * NP, :], y_sb[:])

            if loop_iters == 1:
                body()
            else:
                with tc.For_i(0, loop_iters, 1) as it:
                    body(it)

    nc.compile()
    _CACHE[loop_iters] = nc
    return nc


def prep_inputs(x, w_qkv, w_proj, b_proj):
    """Host-side sharding + layout prep -> per-core input maps."""
    wq, wk, wv = w_qkv[0:DIM], w_qkv[DIM:2 * DIM], w_qkv[2 * DIM:3 * DIM]
    perm = []
    for p in range(PAIRS):
        perm.append(wq[2 * p * D:(2 * p + 2) * D])
        perm.append(wk[2 * p * D:(2 * p + 2) * D])
    wqk_perm = np.concatenate(perm, axis=0)  # [2*DIM, DIM]
    w_cols = np.concatenate([wqk_perm.T, wv.T, w_proj.T], axis=1).astype(BF16)
    bias = b_proj.reshape(1, DIM).astype(BF16)
    in_maps = []
    for i in range(B):
        xt = x[i].T.astype(BF16)
        inp = np.ascontiguousarray(np.concatenate([xt, w_cols], axis=1))
        in_maps.append({"inp": inp, "bias": bias})
    return in_maps


def kernel(x, w_qkv, w_proj, b_proj):
    from concourse import bass_utils

    x = np.asarray(x, dtype=np.float32)
    w_qkv = np.asarray(w_qkv, dtype=np.float32)
    w_proj = np.asarray(w_proj, dtype=np.float32)
    b_proj = np.asarray(b_proj, dtype=np.float32)
    assert x.shape == (B, N, DIM)

    nc = build(1)
    in_maps = prep_inputs(x, w_qkv, w_proj, b_proj)
    res = bass_utils.run_bass_kernel_spmd(nc, in_maps, core_ids=list(range(B)))
    return np.stack([res.results[i]["out"] for i in range(B)], axis=0)



# revision 16
# speedup vs baseline: 1.0934x; 1.0934x over previous
"""Multi-head attention (b=8, n=1024, dim=1024, 16 heads) on 8 TRN2 NeuronCores.

Data-parallel: one batch element per core. Each core runs an identical
Bass/Tile program computing qkv projection, softmax attention, and the
output projection for its [1024, 1024] slice, in bf16 with fp32 PSUM
accumulation.

Layout (host pre-transposes; the device never transposes):
  packA [128, CT, 2304]  per ct-chunk: xt(1024) | wqk pair0 (256) | wvt(1024)
  packC [128, 7, CT, 256] wqk pairs 1-7, pair-major (per-pair DMA granularity)
  wpt   [128, CT, 1024]  w_proj^T, ct-major
  A wqk f-tile 2p / 2p+1 holds the q / k rows of heads 2p,2p+1, so the qkv
  matmul yields q^T/k^T pair tiles with the even head on partitions 0-63 and
  the odd head on 64-127.

Per core:
  q^T,k^T = wqk_perm @ x^T   (4 ct-interleaved PSUM groups for pair 0 so the
            accumulation order matches DMA arrival order at startup)
  V_aug   = x @ wv^T         -> SBUF [n, h*65] with a ones column per head
  S^T     = k_h @ q_h^T      -> PSUM; even/odd heads at PE row groups
            (0,0)/(64,0) run concurrently on HW
  attn^T  = exp(0.125*S^T)   (ScalarE; no max subtraction -- scores ~N(0,1),
            softmax is shift-invariant, exp stays well inside fp32/bf16 range)
  out^T_aug = V_aug^T @ attn^T -> PSUM [65, nq]; row 64 = softmax denominator
  out^T   = out^T_aug[0:64] * recip(denom)  (one gpsimd partition_broadcast
            per pair; odd head shifted to partitions 64-127 via SBUF DMA)
  y       = out_heads @ w_proj^T + bias  -> bf16 out (host converts to fp32)

Schedule: qkv(0) as 4 ct-interleaved PSUM groups (follows DMA arrival);
warmup alternates S(0,nkt)/exp with spacer work (qkv(1) units, V tiles in
the half PSUM pool) so the 2-slot sps rotation never stalls on ACT; then per
pair p: S(p+1)/exp(p+1) + AV(p) + qkv(p+2) slices (+ leftover V in pair 0).
Normalization per pair is batched (4 reciprocals, then 4 partition
broadcasts, then 4 muls) to avoid serializing quadrant chains through the
in-order DVE queue. Input loads are fine-grained per-ct DMAs over the 3 DMA
queues, ordered so the first matmul starts ~2.5us in; outputs are bf16
(host converts back to fp32).
"""

import numpy as np
import ml_dtypes

B, N, DIM = 8, 1024, 1024
H, D = 16, 64
NP = 128  # partitions
NCHUNK = 512  # matmul free-dim chunk (one PSUM bank of fp32)
CT = DIM // NP  # 8 contraction chunks
NT = N // NP  # 8 n-tiles
NQC = N // NCHUNK  # 2 nq chunks
PAIRS = H // 2  # 8 head pairs
AW = 2304  # packA per-ct width: xt 1024 | wqk0 256 | wvt 1024

BF16 = ml_dtypes.bfloat16

_CACHE = {}


def build(loop_iters=1):
    """Build and compile the per-core Bacc graph. Cached per loop_iters."""
    if loop_iters in _CACHE:
        return _CACHE[loop_iters]

    import concourse.mybir as mybir
    import concourse.tile as tile
    from concourse import bacc

    fp32 = mybir.dt.float32
    bf16 = mybir.dt.bfloat16
    Exp = mybir.ActivationFunctionType.Exp

    nc = bacc.Bacc("TRN2", target_bir_lowering=False, debug=False, num_devices=8)

    packA = nc.declare_dram_parameter("packA", [NP, CT * AW], bf16, isOutput=False)
    packC = nc.declare_dram_parameter("packC", [NP, 7 * CT * 256], bf16,
                                      isOutput=False)
    wpt_d = nc.declare_dram_parameter("wpt", [NP, CT * 1024], bf16, isOutput=False)
    bias = nc.declare_dram_parameter("bias", [1, DIM], bf16, isOutput=False)
    out = nc.declare_dram_parameter("out", [N, DIM], bf16, isOutput=True)

    with tile.TileContext(nc) as tc:
        with (
            tc.tile_pool(name="weights", bufs=1) as wpool,
            tc.tile_pool(name="acts", bufs=1) as apool,
            tc.tile_pool(name="attn", bufs=18) as attnpool,
            tc.tile_pool(name="small", bufs=2) as spool,
            tc.tile_pool(name="norm", bufs=4) as npool,
            tc.tile_pool(name="big_ps", bufs=2, space="PSUM") as big_ps,
            tc.tile_pool(name="half_ps", bufs=4, space="PSUM") as half_ps,
        ):
            # ---- persistent SBUF tensors ----
            packA_sb = wpool.tile([NP, CT, AW], bf16, tag="packA")
            packC_sb = wpool.tile([NP, 7, CT, 256], bf16, tag="packC")
            wpt_sb = wpool.tile([NP, CT, 1024], bf16, tag="wpt")
            bias_bc = wpool.tile([NP, DIM], bf16, tag="biasbc")

            # fine-grained input DMAs over 5 queues; first chunk (xt+wqk0 of
            # ct 0) from the otherwise-idle PE queue so qkv can start early.
            qs = [nc.sync, nc.scalar, nc.gpsimd]
            qi = [0]

            def q():
                e = qs[qi[0] % 3]
                qi[0] += 1
                return e

            nc.sync.dma_start(packA_sb[:, 0, 1024:1280], packA[:, 1024:1280])
            nc.scalar.dma_start(packA_sb[:, 0, 0:512], packA[:, 0:512])
            nc.gpsimd.dma_start(packA_sb[:, 0, 512:1024], packA[:, 512:1024])
            for ct in range(1, CT):
                q().dma_start(packA_sb[:, ct, 0:1280],
                              packA[:, ct * AW:ct * AW + 1280])
            q().dma_start(packC_sb[:, 0], packC[:, 0:CT * 256])
            for ct in range(CT):
                q().dma_start(packA_sb[:, ct, 1280:AW],
                              packA[:, ct * AW + 1280:(ct + 1) * AW])
            q().dma_start(bias_bc[:], bias[0:1, :].to_broadcast((NP, DIM)))
            for p in range(2, PAIRS):
                q().dma_start(packC_sb[:, p - 1],
                              packC[:, (p - 1) * CT * 256:p * CT * 256])
            q().dma_start(wpt_sb[:], wpt_d[:])

            xt_sb = packA_sb[:, :, 0:1024]
            wqk0_sb = packA_sb[:, :, 1024:1280]
            wvt_sb = packA_sb[:, :, 1280:AW]

            def wqk_slice(p, ct, which):
                if p == 0:
                    return wqk0_sb[:, ct, which * NP:(which + 1) * NP]
                return packC_sb[:, p - 1, ct, which * NP:(which + 1) * NP]

            def body(_it=None):
                # ---- per-iteration SBUF ----
                q_sb = apool.tile([NP, PAIRS, N], bf16, tag="q")
                k_sb = apool.tile([NP, PAIRS, N], bf16, tag="k")
                vaug_sb = apool.tile([NP, NT, H * (D + 1)], bf16, tag="vaug")
                outT_sb = apool.tile([NP, CT, N], bf16, tag="outT")

                def emit_qkv0():
                    # pair 0, all 4 (which, nqc) PSUM groups ct-interleaved so
                    # the contraction follows DMA arrival order; evacuations
                    # split across DVE and gpsimd.
                    qk = {}
                    for which in range(2):
                        for nqc in range(NQC):
                            qk[which, nqc] = half_ps.tile(
                                [NP, NCHUNK], fp32, tag="half", name="qk0_ps")
                    for ct in range(CT):
                        for which in range(2):
                            for nqc in range(NQC):
                                nc.tensor.matmul(
                                    qk[which, nqc][:],
                                    lhsT=wqk_slice(0, ct, which),
                                    rhs=xt_sb[:, ct,
                                              nqc * NCHUNK:(nqc + 1) * NCHUNK],
                                    start=(ct == 0),
                                    stop=(ct == CT - 1),
                                )
                    # nqc0 halves first (S(0, nkt<4) reads only those);
                    # k0 on ACT || q0 on DVE, then the nqc1 halves.
                    nc.scalar.activation(k_sb[:, 0, 0:NCHUNK], qk[1, 0][:],
                                         mybir.ActivationFunctionType.Copy)
                    nc.vector.tensor_copy(q_sb[:, 0, 0:NCHUNK], qk[0, 0][:])
                    nc.scalar.activation(k_sb[:, 0, NCHUNK:N], qk[1, 1][:],
                                         mybir.ActivationFunctionType.Copy)
                    nc.vector.tensor_copy(q_sb[:, 0, NCHUNK:N], qk[0, 1][:])

                def emit_qkv_ft(p, which, nqc=None):
                    # one f-tile of pair p's qk^T: which=0 -> q, 1 -> k
                    dst = q_sb if which == 0 else k_sb
                    nqcs = range(NQC) if nqc is None else (nqc,)
                    for nqc_ in nqcs:
                        qk_ps = big_ps.tile([NP, NCHUNK], fp32, tag="big",
                                            name="qk_ps")
                        for ct in range(CT):
                            nc.tensor.matmul(
                                qk_ps[:],
                                lhsT=wqk_slice(p, ct, which),
                                rhs=xt_sb[:, ct, nqc_ * NCHUNK:(nqc_ + 1) * NCHUNK],
                                start=(ct == 0),
                                stop=(ct == CT - 1),
                            )
                        nc.vector.tensor_copy(
                            dst[:, p, nqc_ * NCHUNK:(nqc_ + 1) * NCHUNK], qk_ps[:])

                def emit_qkv(p):
                    emit_qkv_ft(p, 0)
                    emit_qkv_ft(p, 1)

                def emit_v_tile(nt, pool="big"):
                    if pool == "big":
                        v_ps = big_ps.tile([NP, 2 * NCHUNK], fp32, tag="big",
                                           name="v_ps")
                        halves = [v_ps[:, fc * NCHUNK:(fc + 1) * NCHUNK]
                                  for fc in range(2)]
                    else:
                        halves = [half_ps.tile([NP, NCHUNK], fp32, tag="half",
                                               name="v_ps")[:]
                                  for fc in range(2)]
                    for ct in range(CT):
                        for fc in range(2):
                            nc.tensor.matmul(
                                halves[fc],
                                lhsT=xt_sb[:, ct, nt * NP:(nt + 1) * NP],
                                rhs=wvt_sb[:, ct, fc * NCHUNK:(fc + 1) * NCHUNK],
                                start=(ct == 0),
                                stop=(ct == CT - 1),
                            )
                    vrow = vaug_sb[:, nt, :].rearrange("p (h e) -> p h e", e=D + 1)
                    nc.vector.memset(vrow[:, :, D:D + 1], 1.0)
                    for fc in range(2):
                        nc.vector.tensor_copy(
                            vrow[:, fc * 8:(fc + 1) * 8, 0:D],
                            halves[fc].rearrange("p (h e) -> p h e", e=D),
                        )

                def emit_s_exp(p, nkt, atn):
                    # S^T for both heads of pair p at nk-tile nkt; even head
                    # on PE rows 0-63, odd on 64-127 (row-tiled, concurrent)
                    sps = {}
                    for hh in range(2):
                        sps[hh] = big_ps.tile([NP, 2 * NCHUNK], fp32,
                                              tag="big", name="s_ps")
                    for nqc in range(NQC):
                        for hh in range(2):
                            lo, hi = hh * D, (hh + 1) * D
                            nc.tensor.matmul(
                                sps[hh][:, nqc * NCHUNK:(nqc + 1) * NCHUNK],
                                lhsT=k_sb[lo:hi, p, nkt * NP:(nkt + 1) * NP],
                                rhs=q_sb[lo:hi, p, nqc * NCHUNK:(nqc + 1) * NCHUNK],
                                start=True,
                                stop=True,
                                tile_position=(hh * D, 0),
                            )
                    for hh in range(2):
                        a = attnpool.tile([NP, N], bf16, tag="attn")
                        nc.scalar.activation(a[:], sps[hh][:], Exp,
                                             scale=float(D) ** -0.5)
                        atn[hh, nkt] = a

                # ---- startup + warmup ----
                # no two S emissions adjacent: V tiles / qkv(1) units between
                # every S so the 2-slot sps rotation never waits on ACT.
                # tokens: Sk = S/exp(0,nkt k), Vk[hb] = v-tile k (half/big
                # pool), Qab = qkv(1) f-tile a, nq-half b.
                import os
                WARMUP = os.environ.get("WARMUP_ORDER",
                    "Q00 S0 Q01 S1 Q10 S2 Q11 S3 V0h S4 V1h S5 V2h S6 V3h "
                    "S7 V4h").split()
                atn_cur = {}
                emit_qkv0()
                for tok in WARMUP:
                    if tok[0] == "S":
                        emit_s_exp(0, int(tok[1]), atn_cur)
                    elif tok[0] == "V":
                        emit_v_tile(int(tok[1]), "big" if tok[2] == "b" else "h")
                    elif tok[0] == "Q":
                        emit_qkv_ft(1, int(tok[1]), int(tok[2]))

                done_v = {int(t[1]) for t in WARMUP if t[0] == "V"}
                rest_v = [v for v in range(NT) if v not in done_v]
                # V(k) must be emitted before AV(0, nkt=k): place at nkt k-3
                pair0_v = {max(0, v - 3) + i_ * 0: v for i_, v in enumerate(rest_v)}
                pair0_v = {}
                nk_slot = 0
                for v in rest_v:
                    nk_slot = max(nk_slot + 1, 1)
                    assert nk_slot <= v, (nk_slot, v)
                    pair0_v[nk_slot] = v

                # ---- pair pipeline ----
                for p in range(PAIRS):
                    av = {}
                    for hh in range(2):
                        for nqc in range(NQC):
                            av[hh, nqc] = half_ps.tile([D + 1, NCHUNK], fp32,
                                                       tag="half", name="av")
                    atn_nxt = {}
                    for nkt in range(NT):
                        if p + 1 < PAIRS:
                            emit_s_exp(p + 1, nkt, atn_nxt)
                        if p == 0 and nkt in pair0_v:
                            emit_v_tile(pair0_v[nkt])
                        if p + 2 < PAIRS and nkt in (1, 3, 5, 7):
                            emit_qkv_ft(p + 2, nkt // 4, (nkt % 4) // 2)
                        for hh in range(2):
                            h = 2 * p + hh
                            for nqc in range(NQC):
                                nc.tensor.matmul(
                                    av[hh, nqc][:],
                                    lhsT=vaug_sb[:, nkt,
                                                 h * (D + 1):(h + 1) * (D + 1)],
                                    rhs=atn_cur[hh, nkt][
                                        :, nqc * NCHUNK:(nqc + 1) * NCHUNK],
                                    start=(nkt == 0),
                                    stop=(nkt == NT - 1),
                                )
                    atn_cur = atn_nxt

                    # normalization: 4 reciprocals gathered into one tile,
                    # ONE partition_broadcast per pair, 4 muls; odd head
                    # partition-shifted 0:64 -> 64:128 via SBUF DMA on the
                    # (idle by now) vector queue.
                    # batched norm: 4 recips, then 4 broadcasts, then 4 muls
                    # (interleaved per-quadrant chains would serialize through
                    # the in-order DVE queue on cross-engine sems)
                    quads = [(hh, nqc) for hh in (1, 0) for nqc in range(NQC)]
                    recips, rbs = {}, {}
                    for hh, nqc in quads:
                        recips[hh, nqc] = npool.tile([1, NCHUNK], fp32,
                                                     tag="recip", name="recip")
                        nc.vector.reciprocal(recips[hh, nqc][:],
                                             av[hh, nqc][D:D + 1, :])
                    for hh, nqc in quads:
                        rbs[hh, nqc] = npool.tile([D, NCHUNK], fp32,
                                                  tag="recipb", name="recipb")
                        nc.gpsimd.partition_broadcast(rbs[hh, nqc][:],
                                                      recips[hh, nqc][:],
                                                      channels=D)
                    tmp = spool.tile([D, N], bf16, tag="tmpodd")
                    for hh, nqc in quads:
                        if hh == 0:
                            dst = outT_sb[0:D, p,
                                          nqc * NCHUNK:(nqc + 1) * NCHUNK]
                        else:
                            dst = tmp[:, nqc * NCHUNK:(nqc + 1) * NCHUNK]
                        nc.vector.tensor_mul(dst, av[hh, nqc][0:D, :],
                                             rbs[hh, nqc][:])
                    nc.sync.dma_start(outT_sb[D:NP, p, :], tmp[:])

                # ---- y = out_heads @ w_proj^T + bias ----
                out_qs = [nc.sync, nc.scalar, nc.gpsimd]
                for nt in range(NT):
                    y_ps = {}
                    for oc in range(NQC):
                        y_ps[oc] = half_ps.tile([NP, NCHUNK], fp32, tag="half",
                                                name="y_ps")
                    for ct in range(CT):
                        for oc in range(NQC):
                            nc.tensor.matmul(
                                y_ps[oc][:],
                                lhsT=outT_sb[:, ct, nt * NP:(nt + 1) * NP],
                                rhs=wpt_sb[:, ct, oc * NCHUNK:(oc + 1) * NCHUNK],
                                start=(ct == 0),
                                stop=(ct == CT - 1),
                            )
                    y_sb = spool.tile([NP, N], bf16, tag="ysb", name="y_sb")
                    for oc in range(NQC):
                        nc.vector.tensor_add(
                            y_sb[:, oc * NCHUNK:(oc + 1) * NCHUNK], y_ps[oc][:],
                            bias_bc[:, oc * NCHUNK:(oc + 1) * NCHUNK],
                        )
                    out_qs[nt % 3].dma_start(
                        out[nt * NP:(nt + 1) * NP, :], y_sb[:])

            if loop_iters == 1:
                body()
            else:
                with tc.For_i(0, loop_iters, 1) as it:
                    body(it)

    nc.compile()
    _CACHE[loop_iters] = nc
    return nc


def _to_ct_major(a):
    """[1024(c), W] -> [128, CT*W]: out[r, ct*W + w] = a[ct*128 + r, w]."""
    W = a.shape[1]
    return np.ascontiguousarray(
        a.reshape(CT, NP, W).transpose(1, 0, 2).reshape(NP, CT * W))


def prep_inputs(x, w_qkv, w_proj, b_proj):
    """Host-side sharding + layout prep -> per-core input maps."""
    wq, wk, wv = w_qkv[0:DIM], w_qkv[DIM:2 * DIM], w_qkv[2 * DIM:3 * DIM]
    wqk0 = _to_ct_major(
        np.concatenate([wq[0:NP], wk[0:NP]], axis=0).T).reshape(NP, CT, 256)
    wvtm = _to_ct_major(wv.T).reshape(NP, CT, 1024)
    packC = np.stack(
        [_to_ct_major(
            np.concatenate([wq[p * NP:(p + 1) * NP],
                            wk[p * NP:(p + 1) * NP]], axis=0).T)
         for p in range(1, PAIRS)],
        axis=1).reshape(NP, 7 * CT * 256).astype(BF16)
    wpt = _to_ct_major(w_proj.T).astype(BF16)
    bias = b_proj.reshape(1, DIM).astype(BF16)
    in_maps = []
    for i in range(B):
        xtm = _to_ct_major(x[i].T).reshape(NP, CT, 1024)
        packA = np.concatenate(
            [xtm, wqk0, wvtm], axis=2).reshape(NP, CT * AW).astype(BF16)
        in_maps.append({"packA": np.ascontiguousarray(packA), "packC": packC,
                        "wpt": wpt, "bias": bias})
    return in_maps


def kernel(x, w_qkv, w_proj, b_proj):
    from concourse import bass_utils

    x = np.asarray(x, dtype=np.float32)
    w_qkv = np.asarray(w_qkv, dtype=np.float32)
    w_proj = np.asarray(w_proj, dtype=np.float32)
    b_proj = np.asarray(b_proj, dtype=np.float32)
    assert x.shape == (B, N, DIM)

    nc = build(1)
    in_maps = prep_inputs(x, w_qkv, w_proj, b_proj)
    res = bass_utils.run_bass_kernel_spmd(nc, in_maps, core_ids=list(range(B)))
    return np.stack(
        [res.results[i]["out"].astype(np.float32) for i in range(B)], axis=0)
